# revision 4
# baseline (speedup 1.0000x reference)
"""GNN message-passing layer on 8 TRN2 NeuronCores.

Math: y[e] = relu(concat(x[i[e]], x[i[e]]) @ W1 + b1) @ W2 + b2
         = relu(x[i[e]] @ (W1[:C]+W1[C:]) + b1) @ W2 + b2.
The MLP depends only on the source node, so compute z = MLP(x) once per
node (50k rows), then y = z[nbr_idx] is a pure gather (800k rows).

Sharding: edges are split evenly across the 8 cores; each core computes
the full z table locally (x + weights replicated; phase A is small) and
then gathers + writes its own edge shard. No collectives.
"""

from contextlib import ExitStack

import numpy as np

import concourse.bacc as bacc
import concourse.bass as bass
import concourse.mybir as mybir
import concourse.tile as tile
from concourse.bass_utils import run_bass_kernel_spmd
from concourse.masks import make_identity

N_CORES = 8
C = 128  # channels (C_IN == C_OUT)
N_NODES = 50000
E_TOTAL = 800000

ACH = 512  # phase-A node chunk (max fp32 moving dim)
NPAD = ((N_NODES + ACH - 1) // ACH) * ACH  # 50176
NCH = NPAD // ACH  # 98

EPC = E_TOTAL // N_CORES  # 100000 edges per core
# one gathered row per partition per indirect DMA (walrus builds one
# descriptor per partition; multi-index-per-partition is NOT unrolled)
SUP = 128  # edges per gather tile
TB = (EPC + SUP - 1) // SUP  # 782 tiles
EPC_PAD = TB * SUP  # 100096

F32 = mybir.dt.float32

# set to mybir.dt.float32r to trade a little matmul precision for speed
MM_DT = mybir.dt.float32


def _build_nc():
    nc = bacc.Bacc("TRN2", target_bir_lowering=False, debug=False,
                   num_devices=N_CORES)

    xT = nc.dram_tensor("xT", [C, NPAD], F32, kind="ExternalInput")
    idx = nc.dram_tensor("idx", [128, TB], mybir.dt.int32,
                         kind="ExternalInput")
    w1 = nc.dram_tensor("w1", [C, C], F32, kind="ExternalInput")
    w2 = nc.dram_tensor("w2", [C, C], F32, kind="ExternalInput")
    b1 = nc.dram_tensor("b1", [C, 1], F32, kind="ExternalInput")
    b2 = nc.dram_tensor("b2", [C, 1], F32, kind="ExternalInput")
    y = nc.dram_tensor("y", [TB * 128, C], F32, kind="ExternalOutput")
    z = nc.dram_tensor("z_table", [NPAD, C], F32)

    with tile.TileContext(nc) as tc, ExitStack() as ctx:
        const = ctx.enter_context(tc.tile_pool(name="const", bufs=1))
        xpool = ctx.enter_context(tc.tile_pool(name="xin", bufs=3))
        hpool = ctx.enter_context(tc.tile_pool(name="hbuf", bufs=3))
        zrow_pool = ctx.enter_context(tc.tile_pool(name="zrow", bufs=6))
        gpool = ctx.enter_context(tc.tile_pool(name="gbuf", bufs=8))
        psA = ctx.enter_context(tc.tile_pool(name="psA", bufs=2, space="PSUM"))
        psT = ctx.enter_context(tc.tile_pool(name="psT", bufs=4, space="PSUM"))

        w1t = const.tile([C, C], F32)
        w2t = const.tile([C, C], F32)
        b1t = const.tile([C, 1], F32)
        b2t = const.tile([C, 1], F32)
        ident = const.tile([128, 128], F32)
        idxt = const.tile([128, TB], mybir.dt.int32)
        nc.sync.dma_start(out=w1t[:], in_=w1[:])
        nc.sync.dma_start(out=w2t[:], in_=w2[:])
        nc.sync.dma_start(out=b1t[:], in_=b1[:])
        nc.sync.dma_start(out=b2t[:], in_=b2[:])
        nc.sync.dma_start(out=idxt[:], in_=idx[:])
        make_identity(nc, ident[:])

        # ---- Phase A: z[n] = relu(x[n] @ W1eff + b1) @ W2 + b2, transposed
        # orientation: zT[:, n-chunk] computed per 512-node chunk, then
        # PE-transposed back to row-major z in DRAM.
        for t in range(NCH):
            xt = xpool.tile([C, ACH], F32)
            nc.sync.dma_start(out=xt[:], in_=xT[:, t * ACH:(t + 1) * ACH])

            h_ps = psA.tile([C, ACH], F32, tag="h_ps")
            nc.tensor.matmul(h_ps[:], w1t[:].bitcast(MM_DT),
                             xt[:].bitcast(MM_DT), start=True, stop=True)
            h_sb = hpool.tile([C, ACH], F32, tag="h_sb")
            nc.scalar.activation(h_sb[:], h_ps[:],
                                 mybir.ActivationFunctionType.Relu,
                                 bias=b1t[:, 0:1])

            z_ps = psA.tile([C, ACH], F32, tag="z_ps")
            nc.tensor.matmul(z_ps[:], w2t[:].bitcast(MM_DT),
                             h_sb[:].bitcast(MM_DT), start=True, stop=True)
            zt_sb = hpool.tile([C, ACH], F32, tag="zt_sb")
            nc.scalar.activation(zt_sb[:], z_ps[:],
                                 mybir.ActivationFunctionType.Identity,
                                 bias=b2t[:, 0:1])

            for b in range(ACH // 128):
                tr_ps = psT.tile([128, 128], F32, tag="tr")
                nc.tensor.transpose(tr_ps[:], zt_sb[:, b * 128:(b + 1) * 128],
                                    ident[:])
                zrow = zrow_pool.tile([128, C], F32, tag="zrow")
                nc.vector.tensor_copy(zrow[:], tr_ps[:])
                n0 = t * ACH + b * 128
                nc.sync.dma_start(out=z[n0:n0 + 128, :], in_=zrow[:])

        tc.strict_bb_all_engine_barrier()

        # ---- Phase B: gather z rows per edge and stream out the y shard.
        for t in range(TB):
            g = gpool.tile([128, C], F32, tag="g")
            nc.gpsimd.indirect_dma_start(
                out=g[:], out_offset=None, in_=z[:],
                in_offset=bass.IndirectOffsetOnAxis(
                    ap=idxt[:, t:t + 1], axis=0))
            nc.sync.dma_start(out=y[t * 128:(t + 1) * 128, :], in_=g[:])

    nc.compile()
    return nc


_NC_CACHE = None


def _get_nc():
    global _NC_CACHE
    if _NC_CACHE is None:
        _NC_CACHE = _build_nc()
    return _NC_CACHE


def kernel(x, nbr_idx, W1, b1, W2, b2, _trace=False, _trace_kwargs=None):
    x = np.asarray(x, dtype=np.float32)
    nbr_idx_np = np.asarray(nbr_idx)
    W1 = np.asarray(W1, dtype=np.float32)
    W2 = np.asarray(W2, dtype=np.float32)
    b1 = np.asarray(b1, dtype=np.float32)
    b2 = np.asarray(b2, dtype=np.float32)

    w1eff = np.ascontiguousarray(W1[:C] + W1[C:])  # [C, C]
    xT = np.zeros((C, NPAD), dtype=np.float32)
    xT[:, :N_NODES] = x.T

    in_maps = []
    for i in range(N_CORES):
        idx_i = nbr_idx_np[i * EPC:(i + 1) * EPC].astype(np.int32)
        idx_pad = np.zeros(EPC_PAD, dtype=np.int32)
        idx_pad[:EPC] = idx_i
        # tile t, partition p  <-  edge t*128 + p
        idx_sb = np.ascontiguousarray(idx_pad.reshape(TB, 128).T)
        in_maps.append({
            "xT": xT,
            "idx": idx_sb,
            "w1": w1eff,
            "w2": W2,
            "b1": b1.reshape(C, 1),
            "b2": b2.reshape(C, 1),
        })

    nc = _get_nc()
    res = run_bass_kernel_spmd(nc, in_maps, list(range(N_CORES)),
                               trace=_trace, **(_trace_kwargs or {}))

    out = np.empty((E_TOTAL, C), dtype=np.float32)
    for i in range(N_CORES):
        out[i * EPC:(i + 1) * EPC] = \
            res.results[i]["y"].reshape(EPC_PAD, C)[:EPC]
    if _trace:
        return out, res
    return out
